# revision 7
# baseline (speedup 1.0000x reference)
"""Ternary CNN forward pass, data-parallel across 8 trn2 NeuronCores.

Sharding: batch dim of x split 8 ways (512 samples/core); all conv/fc
weights replicated. Training-mode BatchNorm uses global batch statistics,
synchronized with a cross-core all-reduce (pmean) of per-device moments
(sync-BN), exactly as the data-parallel decomposition requires.
"""

import numpy as np
import jax
import jax.numpy as jnp

EPS = 1e-5
DELTA = 0.1
N_CORES = 8


def _tern(t, d):
    return jnp.where(t >= d, 1.0, jnp.where(t <= -d, -1.0, 0.0))


def _conv(x, w, stride, pad):
    return jax.lax.conv_general_dilated(
        x, w, window_strides=stride,
        padding=[(pad[0], pad[0]), (pad[1], pad[1])],
        dimension_numbers=('NCHW', 'OIHW', 'NCHW'),
        preferred_element_type=jnp.float32)


def _tconv(x, w, b, stride, pad, first):
    d = DELTA * jnp.max(w)
    tw = _tern(w, d)
    if not first:
        # ternary x ternary matmul is exact in fp16 (products +-1/0,
        # integer sums <= 768, fp32 accumulation) and much faster on PE
        x = _tern(x, d).astype(jnp.float16)
        tw = tw.astype(jnp.float16)
    out = _conv(x, tw, stride, pad)
    return out + _tern(b, d)[None, :, None, None]


def _bn_sync(x, g, b):
    # global (all-shard) batch stats: all-reduce per-device moments
    m = jax.lax.pmean(jnp.mean(x, axis=(0, 2, 3)), 'i')
    m2 = jax.lax.pmean(jnp.mean(x * x, axis=(0, 2, 3)), 'i')
    v = m2 - m * m
    m = m[None, :, None, None]
    v = v[None, :, None, None]
    return g[None, :, None, None] * (x - m) * jax.lax.rsqrt(v + EPS) \
        + b[None, :, None, None]


def _maxpool(x, k, s):
    return jax.lax.reduce_window(x, -jnp.inf, jax.lax.max,
                                 (1, 1, k[0], k[1]), (1, 1, s[0], s[1]),
                                 'VALID')


def _ht(x):
    return jnp.clip(x, -1.0, 1.0)


def _fwd(x, w1, b1, g1, bb1, w2, b2, g2, bb2, w3, b3, g3, bb3,
         w4, b4, g4, bb4, fcw, fcb):
    h = _tconv(x, w1, b1, (1, 2), (0, 4), first=True)
    h = _ht(_bn_sync(h, g1, bb1))
    h = _maxpool(h, (1, 2), (1, 2))
    h = _tconv(h, w2, b2, (1, 1), (0, 1), first=False)
    h = _ht(_bn_sync(h, g2, bb2))
    h = _tconv(h, w3, b3, (1, 1), (0, 1), first=False)
    h = _ht(_bn_sync(h, g3, bb3))
    h = _maxpool(h, (1, 2), (1, 2))
    h = _tconv(h, w4, b4, (1, 1), (0, 0), first=False)
    h = _ht(_bn_sync(h, g4, bb4))
    h = h.reshape(h.shape[0], -1)
    d = DELTA * jnp.max(fcw)
    hq = _tern(h, d).astype(jnp.float16)
    tfcw = _tern(fcw, d).astype(jnp.float16)
    out = jnp.matmul(hq, tfcw.T, preferred_element_type=jnp.float32) \
        + _tern(fcb, d)[None, :]
    return out


_WNAMES = ['w1', 'b1', 'g1', 'bb1', 'w2', 'b2', 'g2', 'bb2',
           'w3', 'b3', 'g3', 'bb3', 'w4', 'b4', 'g4', 'bb4', 'fcw', 'fcb']

_pfwd = None


def _get_pfwd():
    global _pfwd
    if _pfwd is None:
        _pfwd = jax.pmap(
            _fwd, axis_name='i',
            in_axes=(0,) + (None,) * len(_WNAMES),
            devices=jax.devices()[:N_CORES])
    return _pfwd


def kernel(**inputs):
    x = np.asarray(inputs['x'], dtype=np.float32)
    B = x.shape[0]
    shard = B // N_CORES
    # NOTE: x must stay fp32 — the ternary net avalanche-amplifies input
    # rounding (bf16 x measured rel err 0.46 vs 1.9e-3 fp32)
    xs = x.reshape(N_CORES, shard, *x.shape[1:])
    ws = [np.asarray(inputs[n], dtype=np.float32) for n in _WNAMES]
    out = _get_pfwd()(xs, *ws)
    out = np.asarray(out, dtype=np.float32).reshape(B, -1)
    return out



# revision 9
# speedup vs baseline: 1.0329x; 1.0329x over previous
"""Ternary CNN forward pass, data-parallel across 8 trn2 NeuronCores.

Sharding: batch dim of x split 8 ways (512 samples/core); all conv/fc
weights replicated. Training-mode BatchNorm uses global batch statistics,
synchronized with a cross-core all-reduce (pmean) of per-device moments
(sync-BN), exactly as the data-parallel decomposition requires.
"""

import numpy as np
import jax
import jax.numpy as jnp

EPS = 1e-5
DELTA = 0.1
N_CORES = 8


def _tern(t, d):
    return jnp.where(t >= d, 1.0, jnp.where(t <= -d, -1.0, 0.0))


def _conv(x, w, stride, pad):
    return jax.lax.conv_general_dilated(
        x, w, window_strides=stride,
        padding=[(pad[0], pad[0]), (pad[1], pad[1])],
        dimension_numbers=('NCHW', 'OIHW', 'NCHW'))


def _tconv(x, w, b, stride, pad, first):
    d = DELTA * jnp.max(w)
    if not first:
        x = _tern(x, d)
    out = _conv(x, _tern(w, d), stride, pad)
    return out + _tern(b, d)[None, :, None, None]


def _bn_sync(x, g, b):
    # global (all-shard) batch stats: all-reduce per-device moments
    m = jax.lax.pmean(jnp.mean(x, axis=(0, 2, 3)), 'i')
    m2 = jax.lax.pmean(jnp.mean(x * x, axis=(0, 2, 3)), 'i')
    v = m2 - m * m
    m = m[None, :, None, None]
    v = v[None, :, None, None]
    return g[None, :, None, None] * (x - m) * jax.lax.rsqrt(v + EPS) \
        + b[None, :, None, None]


def _maxpool(x, k, s):
    return jax.lax.reduce_window(x, -jnp.inf, jax.lax.max,
                                 (1, 1, k[0], k[1]), (1, 1, s[0], s[1]),
                                 'VALID')


def _ht(x):
    return jnp.clip(x, -1.0, 1.0)


def _fwd(x, w1, b1, g1, bb1, w2, b2, g2, bb2, w3, b3, g3, bb3,
         w4, b4, g4, bb4, fcw, fcb):
    h = _tconv(x, w1, b1, (1, 2), (0, 4), first=True)
    h = _ht(_bn_sync(h, g1, bb1))
    h = _maxpool(h, (1, 2), (1, 2))
    h = _tconv(h, w2, b2, (1, 1), (0, 1), first=False)
    h = _ht(_bn_sync(h, g2, bb2))
    h = _tconv(h, w3, b3, (1, 1), (0, 1), first=False)
    h = _ht(_bn_sync(h, g3, bb3))
    h = _maxpool(h, (1, 2), (1, 2))
    h = _tconv(h, w4, b4, (1, 1), (0, 0), first=False)
    h = _ht(_bn_sync(h, g4, bb4))
    h = h.reshape(h.shape[0], -1)
    d = DELTA * jnp.max(fcw)
    hq = _tern(h, d)
    out = hq @ _tern(fcw, d).T + _tern(fcb, d)[None, :]
    return out


_WNAMES = ['w1', 'b1', 'g1', 'bb1', 'w2', 'b2', 'g2', 'bb2',
           'w3', 'b3', 'g3', 'bb3', 'w4', 'b4', 'g4', 'bb4', 'fcw', 'fcb']

_pfwd = None


def _get_pfwd():
    global _pfwd
    if _pfwd is None:
        _pfwd = jax.pmap(
            _fwd, axis_name='i',
            in_axes=(0,) + (None,) * len(_WNAMES),
            devices=jax.devices()[:N_CORES])
    return _pfwd


def kernel(**inputs):
    x = np.asarray(inputs['x'], dtype=np.float32)
    B = x.shape[0]
    shard = B // N_CORES
    # NOTE: x must stay fp32 — the ternary net avalanche-amplifies input
    # rounding (bf16 x measured rel err 0.46 vs 1.9e-3 fp32; fp16/int8
    # quantization of x fails the same way)
    xs = x.reshape(N_CORES, shard, *x.shape[1:])
    ws = [np.asarray(inputs[n], dtype=np.float32) for n in _WNAMES]
    out = _get_pfwd()(xs, *ws)
    out = np.asarray(out, dtype=np.float32).reshape(B, -1)
    return out

